# revision 1
# baseline (speedup 1.0000x reference)
"""SpecAugment (log-mel masking) Trainium2 kernel.

Full inputs: x [64,128,3000] f32, f0/f_w/t0/t_w [64,2] i32.
out[b,f,t] = fill_b if (f in freq band) or (t in time band) else x[b,f,t],
fill_b = min over x[b].

Strategy: batch-shard B=64 across 8 cores (8 samples/core). The int mask
params are tiny host tensors, so the per-sample 0/1 mask vectors are
computed on host and shipped as bf16 data; the device does only the
memory-bound work. Per sample:
  - DMA x[b] [128,3000] -> SBUF
  - DVE reduce_min (free axis) -> [128,1]; tiny DMA gather -> [1,128];
    reduce_min -> fill [1,1]; broadcast to [128,1] via tiny PE matmul
  - combined mask = ones(x)mt + mf(x)ones as ONE K=2 bf16 matmul per
    512-col chunk into PSUM (values {0,1,2}; nonzero == masked)
  - DVE copy_predicated overwrites masked cells with fill (data operand
    is fill128 broadcast along the free axis)
  - DMA xt -> y[b]
HBM traffic is the minimum 2 x 12.3MB per core -> ~69us roofline.
"""

import ml_dtypes
import numpy as np

import concourse.bacc as bacc
import concourse.bass as bass
import concourse.mybir as mybir
import concourse.tile as tile
import concourse.bass_utils as bass_utils

B, F, T = 64, 128, 3000
N_CORES = 8
BPC = B // N_CORES  # samples per core
F32 = mybir.dt.float32
BF16 = mybir.dt.bfloat16

_cached = {}


def _build_nc():
    nc = bacc.Bacc("TRN2", target_bir_lowering=False, debug=False)
    x = nc.dram_tensor("x_sh", [BPC, F, T], F32, kind="ExternalInput")
    # row0 = time mask (0/1), row1 = ones
    mtr = nc.dram_tensor("mtr_sh", [BPC, 2, T], BF16, kind="ExternalInput")
    # row0 = ones, row1 = freq mask (0/1)
    mfl = nc.dram_tensor("mfl_sh", [BPC, 2, F], BF16, kind="ExternalInput")
    y = nc.dram_tensor("y_sh", [BPC, F, T], F32, kind="ExternalOutput")

    xa, ta, fa, ya = x.ap(), mtr.ap(), mfl.ap(), y.ap()

    H = T // 2

    with tile.TileContext(nc) as tc:
        with (
            tc.tile_pool(name="xp", bufs=6) as xp,
            tc.tile_pool(name="row", bufs=6) as rowp,
            tc.tile_pool(name="small", bufs=6) as sp,
            tc.tile_pool(name="single", bufs=1) as single,
            tc.tile_pool(name="ps", bufs=2, space="PSUM") as psp,
            tc.tile_pool(name="ps_small", bufs=2, space="PSUM") as psps,
        ):
            ones_row = single.tile([1, F], F32)
            nc.vector.memset(ones_row, 1.0)
            one11 = single.tile([1, 1], F32)
            nc.vector.memset(one11, 1.0)

            for b in range(BPC):
                xt = xp.tile([F, T], F32, tag="xt")
                nc.sync.dma_start(out=xt, in_=xa[b])
                mtb = rowp.tile([2, T], BF16, tag="mtb")
                nc.gpsimd.dma_start(out=mtb, in_=ta[b])
                mfb = sp.tile([2, F], BF16, tag="mfb")
                nc.gpsimd.dma_start(out=mfb, in_=fa[b])

                # combined mask first: PE work depends only on mtb/mfb,
                # so it overlaps the reduce chain below
                ms_halves = []
                for h in range(2):
                    msh = psp.tile([F, H], F32, tag="ms")
                    for c0 in range(0, H, 512):
                        cw = min(512, H - c0)
                        nc.tensor.matmul(
                            msh[:, c0 : c0 + cw],
                            mfb,
                            mtb[:, h * H + c0 : h * H + c0 + cw],
                            start=True,
                            stop=True,
                        )
                    ms_halves.append(msh)

                # per-sample min: free-axis reduce, gather across partitions
                colmin = sp.tile([F, 1], F32, tag="colmin")
                nc.vector.tensor_reduce(
                    out=colmin, in_=xt, axis=mybir.AxisListType.X,
                    op=mybir.AluOpType.min,
                )
                rowmin = sp.tile([1, F], F32, tag="rowmin")
                nc.gpsimd.dma_start(out=rowmin, in_=colmin)
                fill11 = sp.tile([1, 1], F32, tag="fill11")
                nc.vector.tensor_reduce(
                    out=fill11, in_=rowmin, axis=mybir.AxisListType.X,
                    op=mybir.AluOpType.min,
                )
                # fill broadcast [1,1] -> [1,128] (free) -> [128,1] (PE)
                fill_row = sp.tile([1, F], F32, tag="fill_row")
                nc.scalar.mul(fill_row, ones_row, fill11)
                fill128_ps = psps.tile([F, 1], F32, tag="fill128_ps")
                nc.tensor.matmul(fill128_ps, fill_row, one11, start=True, stop=True)
                fill128 = sp.tile([F, 1], F32, tag="fill128")
                nc.scalar.copy(fill128, fill128_ps)

                # nonzero mask => masked cell; overwrite with fill, then
                # store each half as soon as its pred completes
                for h in range(2):
                    nc.vector.copy_predicated(
                        out=xt[:, h * H : (h + 1) * H],
                        mask=ms_halves[h].bitcast(mybir.dt.int32),
                        data=fill128.to_broadcast([F, H]),
                    )
                    nc.scalar.dma_start(
                        out=ya[b][:, h * H : (h + 1) * H],
                        in_=xt[:, h * H : (h + 1) * H],
                    )
    nc.compile()
    return nc


def _host_masks(f0, f_w, t0, t_w):
    nb = f0.shape[0]
    fidx = np.arange(F, dtype=np.int32)
    tidx = np.arange(T, dtype=np.int32)
    fm = (
        (fidx[None, None, :] >= f0[:, :, None])
        & (fidx[None, None, :] < (f0 + f_w)[:, :, None])
    ).any(axis=1)  # [B,F] bool
    tm = (
        (tidx[None, None, :] >= t0[:, :, None])
        & (tidx[None, None, :] < (t0 + t_w)[:, :, None])
    ).any(axis=1)  # [B,T] bool
    mtr = np.ones((nb, 2, T), np.float32)
    mtr[:, 0, :] = tm
    mfl = np.ones((nb, 2, F), np.float32)
    mfl[:, 1, :] = fm
    return mtr.astype(ml_dtypes.bfloat16), mfl.astype(ml_dtypes.bfloat16)


def kernel(x, f0, f_w, t0, t_w, **_):
    x = np.ascontiguousarray(np.asarray(x, dtype=np.float32))
    f0 = np.asarray(f0)
    f_w = np.asarray(f_w)
    t0 = np.asarray(t0)
    t_w = np.asarray(t_w)
    mtr, mfl = _host_masks(f0, f_w, t0, t_w)

    if "nc" not in _cached:
        _cached["nc"] = _build_nc()
    nc = _cached["nc"]

    in_maps = []
    for c in range(N_CORES):
        s = slice(c * BPC, (c + 1) * BPC)
        in_maps.append(
            {
                "x_sh": np.ascontiguousarray(x[s]),
                "mtr_sh": np.ascontiguousarray(mtr[s]),
                "mfl_sh": np.ascontiguousarray(mfl[s]),
            }
        )
    res = bass_utils.run_bass_kernel_spmd(
        nc, in_maps, core_ids=list(range(N_CORES))
    )
    out = np.concatenate([r["y_sh"] for r in res.results], axis=0)
    return out

